# revision 16
# baseline (speedup 1.0000x reference)
"""Self-attention (SAGAN-style, spectral-normalized 1x1 convs) on 8 TRN2 cores.

Contract: kernel(**inputs) takes the FULL unsharded inputs
(x [8,512,64,64], weights, power-iteration u vectors, gamma) and returns
the FULL output [8,512,64,64] (float32).

Sharding: data-parallel over batch B=8 -> one batch element per core.
Each core runs the complete attention block for its element; no
collectives are needed.

Per-core math (C=512, HW=4096, M=HW/4=1024):
    theta = sn(w_theta) @ x          [64, 4096]
    phi   = maxpool2(sn(w_phi) @ x)  [64, 1024]
    g     = maxpool2(sn(w_g)   @ x)  [256, 1024]
    sT[m,n] = sum_c phi[c,m] theta[c,n]
    beta  = softmax over m  (exp without max-subtraction: logits span
            ~+-51, safe in fp32/bf16; normalization applied to o)
    o     = (g @ exp(sT)) * (1/sum)  [256, 4096]
    out   = gamma * (sn(w_o) @ o) + x

Precision plan (measured 1.07e-2 output rel err vs the 2e-2 gate):
 - logit path fp16 (x16, theta, phi); E/g/gT bf16; fp32 PSUM
 - g-projection fp8 e4m3 DoubleRow (k=256/matmul); wg pre-scaled by 8
   (e4m3-normal range), compensated exactly by the 8.0-valued
   ones-matmul that makes the softmax denominators
 - o stored e4m3; out-projection fp8 e4m3 DoubleRow with wo pre-scaled
   by 512*gamma; 1/512 rides in the residual ops' free scale slots
 - residual + output fp16 (host upcasts to fp32)

Schedule notes (from perfetto/ntff analysis of earlier versions):
 - FD=512 matmuls pipeline at ~216ns start-to-start (incl implicit
   LDWEIGHTS), so wall time ~ matmul slot count; both big k>=256
   contractions (g-proj, out-proj) use fp8 DoubleRow to halve slots.
 - sT matmuls are plain full-row k=128: theta/phi partition rows 64:128
   are ZEROED once, so each k=64 product rides a full matmul without
   tile_position packing (measured: packed pairs serialize their
   LDWEIGHTS on row-group conflicts and cost ~2.3x a plain slot).
 - exp starts as soon as theta/phi are done (~16us): sT units for
   blocks 0-3 are WOVEN between the g-projection/transpose blocks, so
   the 36us ACT exp stream (the #2 floor of this kernel) overlaps the
   whole projection tail. x16 rides sync+scalar DMA rings, x8+weights
   ride the gpsimd ring in parallel; all DMA dispatches lead the program.
 - maxpool is 2-stage (one 1x PSUM-read max + one 2x SBUF max) instead
   of copy+3 maxes: DVE proj load drops ~2.5x.
 - softmax sums: 3-level DVE tree + ONE 8.0-ones-matmul per block;
   reciprocal_approx_fast; sums sit between the two o-matmul halves.
 - out-projection of block nb-1 is emitted split: PE matmuls + 2 DVE
   fused residuals early (fills exp waits), ACT copy-scale + GPSIMD
   adds + one out-DMA per block later.

PE->PE self-waits are stripped (PE->PSUM write port is FIFO) and bacc's
generate_event_semaphores legalizes the 1-wait ISA limit.

The spectral-norm power-iteration only involves [1,64]x[64,512]
matvecs, so it runs on the host in float32; gamma is folded into w_o.
"""

import numpy as np

B, C, H, W = 8, 512, 64, 64
HW = H * W            # 4096
M = HW // 4           # 1024 (pooled spatial)
C8 = C // 8           # 64
C2 = C // 2           # 256
P = 128               # SBUF partitions
KC = C // P           # 4 k-chunks for C-contraction
FB = 512              # free-dim block
NB = HW // FB         # 8 n-blocks
MC = M // P           # 8 m-chunks
WG_SCALE = 8.0        # host wg scale (e4m3 normal range), cancelled by
                      # using this value in the ones-matmul
WO_SCALE = 512.0      # host wo scale (e4m3 normal range), cancelled in
                      # the residual ops' scale slots
EPS = 1e-12

_CACHE = {}


def _sn(w, u):
    """Host-side spectral norm (eval-mode power iteration), float32."""
    w = np.asarray(w, np.float32)
    u = np.asarray(u, np.float32)
    v = u @ w
    v = v / max(np.float32(np.linalg.norm(v)), np.float32(EPS))
    u2 = v @ w.T
    u2 = u2 / max(np.float32(np.linalg.norm(u2)), np.float32(EPS))
    sv = np.float32((v @ w.T @ u2.T)[0, 0])
    return w / sv


def _strip_pe_self_waits(nc):
    """Remove S[PE]-waits from PE matmuls: PE->PE deps are ordered by the
    engine queue + FIFO PSUM write port, and matmuls only have one ISA
    wait slot."""
    import concourse.mybir as mybir

    for f in nc.m.functions:
        for blk in f.blocks:
            for inst in blk.instructions:
                if not isinstance(inst, mybir.InstMatmult):
                    continue
                si = inst.sync_info
                kept = [w for w in si.on_wait
                        if not (w.ant_name or "").startswith("PE_")]
                if len(kept) != len(si.on_wait):
                    si.on_wait = kept
                    inst.sync_info = si


def _build_nc():
    import concourse.bass as bass
    import concourse.mybir as mybir
    import concourse.tile as tile
    from concourse import bacc
    from concourse.masks import make_identity

    fp32 = mybir.dt.float32
    fp16 = mybir.dt.float16
    bf16 = mybir.dt.bfloat16
    fp8 = mybir.dt.float8e4
    DR = mybir.MatmulPerfMode.DoubleRow
    Exp = mybir.ActivationFunctionType.Exp
    mult = mybir.AluOpType.mult
    add = mybir.AluOpType.add
    mx = mybir.AluOpType.max

    nc = bacc.Bacc()
    x16_d = nc.dram_tensor("x16", [C, HW], fp16, kind="ExternalInput").ap()
    x8_d = nc.dram_tensor("x8", [C, HW], fp8, kind="ExternalInput").ap()
    wtp_d = nc.dram_tensor("wtp", [C, P], fp16, kind="ExternalInput").ap()
    wg_d = nc.dram_tensor("wg", [C, C2], fp8, kind="ExternalInput").ap()
    wo_d = nc.dram_tensor("wo", [C2, C], fp8, kind="ExternalInput").ap()
    out_d = nc.dram_tensor("out", [C, HW], fp16, kind="ExternalOutput").ap()

    x16_r = x16_d.rearrange("(kc p) n -> p kc n", p=P)
    x8_r = x8_d.rearrange("(kc p) n -> p kc n", p=P)
    out_r = out_d.rearrange("(ig p) n -> p ig n", p=P)

    with tile.TileContext(nc) as tc:
        with (
            tc.tile_pool(name="sb", bufs=1) as sb,
            tc.tile_pool(name="psS", bufs=2, space="PSUM") as psS,
        ):
            # ---- persistent tiles ----
            x16 = sb.tile([P, KC, HW], fp16)
            x8 = sb.tile([P, KC, HW], fp8)
            wtp = sb.tile([P, KC, P], fp16)
            wg8 = sb.tile([P, KC, C2], fp8)
            wo8 = sb.tile([P, 2, C], fp8)
            theta_sb = sb.tile([P, HW], fp16)             # rows 64:128 zero
            phi2 = sb.tile([P, NB, 4, 32], fp16)          # rows 64:128 zero
            g2 = sb.tile([P, 2, M], bf16)                 # pooled, = 8*g
            gT_sb = sb.tile([P, MC, C2], bf16)            # [m-part, mc, c]
            identity = sb.tile([P, P], bf16)
            ones_mat = sb.tile([P, P], bf16)              # value WG_SCALE

            phi_flat = phi2.rearrange("p a b c -> p (a b c)")
            g4 = g2.rearrange("p cg (fb h2 w2) -> p cg fb h2 w2", h2=4, w2=32)

            # ---- input DMAs lead the program: x16 on the sync+scalar
            # rings, x8 + weights on the gpsimd ring (parallel) ----
            nc.sync.dma_start(wtp, wtp_d.rearrange("(kc p) i -> p kc i", p=P))
            for q in range(NB):
                sl = slice(q * FB, (q + 1) * FB)
                eng = nc.sync if q % 2 == 0 else nc.scalar
                eng.dma_start(x16[:, :, sl], x16_r[:, :, sl])
            nc.gpsimd.dma_start(wg8, wg_d.rearrange("(kc p) i -> p kc i", p=P))
            nc.gpsimd.dma_start(wo8, wo_d.rearrange("(cg p) i -> p cg i", p=P))
            for q in range(NB // 2):
                fsl = slice(2 * q * FB, (2 * q + 2) * FB)
                nc.gpsimd.dma_start(x8[:, :, fsl], x8_r[:, :, fsl])

            # ---- constants; zero the unused partition halves so sT can
            # run plain full-row k=128 matmuls ----
            nc.vector.memset(ones_mat, WG_SCALE)
            nc.vector.memset(theta_sb[C8:, :], 0.0)
            nc.vector.memset(phi2[C8:], 0.0)
            with tc.tile_pool(name="psW", bufs=1, space="PSUM") as psW:
                # junk matmuls keep the PE busy during the DMA head so the
                # HAM clock-gate opens before the real matmul stream
                wps = psW.tile([P, P], fp32, tag="warm")
                for _ in range(10):
                    nc.tensor.matmul(wps, lhsT=ones_mat, rhs=ones_mat,
                                     start=True, stop=True)
            ident_raw = sb.tile([P, P], fp32)
            make_identity(nc, ident_raw)
            nc.scalar.copy(identity, ident_raw)

            # ---------- attention front half: sT unit emitter ----------
            # expT tiles for blocks 0-3 are produced during the projection
            # weave; bufs=5 keeps enough alive until their o-matmuls run.
            exp_tiles = {}

            def st_pair(nb, mc2):
                """Two sT chunks: full-row k=128 matmuls (rows 64:128 are
                zeros) into one PSUM tile + a single FD-1024 exp."""
                if mc2 == 0:
                    exp_tiles[nb] = sb.tile([P, MC, FB], bf16, tag="expT",
                                            bufs=5, name=f"expT{nb}")
                expT = exp_tiles[nb]
                nsl = slice(nb * FB, (nb + 1) * FB)
                ps = psS.tile([P, 2, FB], fp32, tag="sT")
                for j in range(2):
                    mc = 2 * mc2 + j
                    nc.tensor.matmul(
                        ps[:, j, :],
                        lhsT=phi_flat[:, mc * P:(mc + 1) * P],
                        rhs=theta_sb[:, nsl],
                        start=True, stop=True,
                    )
                nc.scalar.activation(
                    expT[:, 2 * mc2:2 * mc2 + 2, :].rearrange("p a b -> p (a b)"),
                    ps.rearrange("p a b -> p (a b)"), Exp,
                )

            def pool2(ps2, dst, base, dt):
                """3-op 2x2 maxpool of a [*, 2, FB] PSUM view into dst
                [*, 2, 4, 32]: PSUM copy (wr=0), max vs PSUM (wr=1) -- only
                one PSUM operand per op -- then a 2x-rate SBUF hr-max."""
                v = ps2.rearrange("p a (h2 hr w2 wr) -> p a h2 hr w2 wr",
                                  hr=2, w2=32, wr=2)
                s1 = sb.tile([P, 2, 4, 2, 32], dt, tag="pool1", bufs=3,
                             name="s1")[base:]
                nc.vector.tensor_copy(s1, v[:, :, :, :, :, 0])
                nc.vector.tensor_tensor(s1, s1, v[:, :, :, :, :, 1], mx)
                nc.vector.tensor_tensor(dst, s1[:, :, :, 0, :],
                                        s1[:, :, :, 1, :], mx)

            # ---------- theta/phi projection (x16-chunk paced) ----------
            with tc.tile_pool(name="psB", bufs=2, space="PSUM") as psB:
                for fb2 in range(NB // 2):
                    ps = psB.tile([P, 2, FB], fp32, tag="proj", name="ps")
                    for half in range(2):
                        for kc in range(KC):
                            nc.tensor.matmul(
                                ps[:, half, :],
                                lhsT=wtp[:, kc, :],
                                rhs=x16[:, kc, (2 * fb2 + half) * FB:(2 * fb2 + half + 1) * FB],
                                start=(kc == 0), stop=(kc == KC - 1),
                            )
                    th = ps[:C8].rearrange("p a b -> p (a b)")
                    thsl = slice(2 * fb2 * FB, (2 * fb2 + 2) * FB)
                    nc.scalar.copy(theta_sb[:C8, thsl], th)
                    pool2(ps[C8:], phi2[:C8, 2 * fb2:2 * fb2 + 2], C8, fp16)

            # ---------- g-projection + transposes, WOVEN with the sT
            # pairs of blocks 0-3 so the exp stream starts immediately;
            # the single-buffered g pool's wait-for-pool gaps are filled
            # by the sT matmuls ----------
            with (
                tc.tile_pool(name="psA", bufs=1, space="PSUM") as psA,
                tc.tile_pool(name="psT", bufs=1, space="PSUM") as psT,
            ):
                def g_block(fb2, cg):
                    ps = psA.tile([P, 2, FB], fp32, tag="proj", name="psg")
                    for half in range(2):
                        hsl = slice((2 * fb2 + half) * FB,
                                    (2 * fb2 + half + 1) * FB)
                        for qq in range(2):
                            nc.tensor.matmul(
                                ps[:, half, :],
                                lhsT=wg8[:, 2 * qq:2 * qq + 2, cg * P:(cg + 1) * P],
                                rhs=x8[:, 2 * qq:2 * qq + 2, hsl],
                                start=(qq == 0), stop=(qq == 1),
                                perf_mode=DR,
                            )
                    pool2(ps, g4[:, cg, 2 * fb2:2 * fb2 + 2], 0, bf16)

                def tr_block(fb2):
                    for mc in (2 * fb2, 2 * fb2 + 1):
                        pt = psT.tile([P, 2, P], bf16, tag="tr")
                        for cg in range(2):
                            nc.tensor.transpose(
                                pt[:, cg, :], g2[:, cg, mc * P:(mc + 1) * P],
                                identity,
                            )
                        nc.scalar.copy(gT_sb[:, mc, :],
                                       pt.rearrange("p a b -> p (a b)"))

                pairs = [(nb, mc2) for nb in range(4) for mc2 in range(MC // 2)]
                ui = 0
                for fb2 in range(NB // 2):
                    for cg in range(2):
                        for _ in range(2):
                            st_pair(*pairs[ui]); ui += 1
                        g_block(fb2, cg)
                    tr_block(fb2)
                assert ui == len(pairs)

            # ---------- attention back half ----------
            with (
                tc.tile_pool(name="psO", bufs=2, space="PSUM") as psO,
                tc.tile_pool(name="psO2", bufs=2, space="PSUM") as psO2,
            ):
                pending = []

                def _emit_o2_early(item):
                    """Out-projection matmuls of block jnb (4 fp8-DR, k=256)
                    + the 2 DVE fused residuals."""
                    jnb, jo8 = item
                    jsl = slice(jnb * FB, (jnb + 1) * FB)
                    ot4 = sb.tile([P, 4, FB], fp16, tag="out", bufs=2,
                                  name="ot4")
                    o2s = []
                    for ig in range(4):
                        o2 = psO2.tile([P, FB], fp32, tag="o2", name="o2")
                        nc.tensor.matmul(
                            o2,
                            lhsT=wo8[:, :, ig * P:(ig + 1) * P],
                            rhs=jo8,
                            start=True, stop=True, perf_mode=DR,
                        )
                        if ig < 2:
                            nc.vector.scalar_tensor_tensor(
                                ot4[:, ig, :], o2, 1.0 / WO_SCALE,
                                x16[:, ig, jsl], op0=mult, op1=add)
                        else:
                            o2s.append(o2)
                    return (jnb, ot4, o2s)

                def _emit_o2_late(item):
                    """ACT copy-with-scale + GPSIMD residual adds for ig 2,3
                    and the single out-DMA."""
                    jnb, ot4, o2s = item
                    jsl = slice(jnb * FB, (jnb + 1) * FB)
                    for ig, o2 in zip((2, 3), o2s):
                        st = sb.tile([P, FB], fp32, tag="stage", bufs=4,
                                     name="st")
                        nc.scalar.mul(st, o2, 1.0 / WO_SCALE)
                        nc.gpsimd.tensor_tensor(ot4[:, ig, :], st,
                                                x16[:, ig, jsl], add)
                    nc.sync.dma_start(out_r[:, :, jsl], ot4)

                for nb in range(NB):
                    held = None
                    if nb >= 4:
                        st_pair(nb, 0)
                        st_pair(nb, 1)
                        held = _emit_o2_early(pending.pop(0)) if pending else None
                        st_pair(nb, 2)
                        st_pair(nb, 3)
                    else:
                        held = _emit_o2_early(pending.pop(0)) if pending else None
                    expT = exp_tiles.pop(nb)

                    # sum over m: 3-level DVE tree, then ONE ones-matmul
                    # (value 8.0) sums the partitions; output rows all hold
                    # the sum -> broadcast-ready
                    part = sb.tile([P, 4, FB], bf16, tag="part", bufs=2)
                    nc.vector.tensor_tensor(
                        part.rearrange("p a b -> p (a b)"),
                        expT[:, 0:4, :].rearrange("p a b -> p (a b)"),
                        expT[:, 4:8, :].rearrange("p a b -> p (a b)"), add)
                    nc.vector.tensor_tensor(
                        part[:, 0:2, :].rearrange("p a b -> p (a b)"),
                        part[:, 0:2, :].rearrange("p a b -> p (a b)"),
                        part[:, 2:4, :].rearrange("p a b -> p (a b)"), add)
                    nc.vector.tensor_tensor(part[:, 0, :], part[:, 0, :],
                                            part[:, 1, :], add)

                    # o[c, n] = sum_m gT[m, c] expT[m, n]; the sum-matmul
                    # sits between the cg halves so the reciprocal is ready
                    # when the first half finishes accumulating
                    o8_sb = sb.tile([P, 2, FB], fp8, tag="o8", bufs=2)
                    o_ps0 = psO.tile([P, FB], fp32, tag="o_ps", name="o0")
                    for mc in range(MC):
                        nc.tensor.matmul(
                            o_ps0,
                            lhsT=gT_sb[:, mc, 0:P],
                            rhs=expT[:, mc, :],
                            start=(mc == 0), stop=(mc == MC - 1),
                        )
                    sum_ps = psS.tile([P, 2, FB], fp32, tag="sT",
                                      name="sum_ps")[:, 0, :]
                    nc.tensor.matmul(sum_ps, lhsT=ones_mat, rhs=part[:, 0, :],
                                     start=True, stop=True)
                    recipb = sb.tile([P, FB], fp32, tag="recipb", bufs=2)
                    nc.vector.reciprocal_approx_fast(recipb, sum_ps)
                    o_ps1 = psO.tile([P, FB], fp32, tag="o_ps", name="o1")
                    for mc in range(MC):
                        nc.tensor.matmul(
                            o_ps1,
                            lhsT=gT_sb[:, mc, P:C2],
                            rhs=expT[:, mc, :],
                            start=(mc == 0), stop=(mc == MC - 1),
                        )
                    nc.vector.tensor_tensor(o8_sb[:, 0, :], o_ps0, recipb, mult)
                    nc.vector.tensor_tensor(o8_sb[:, 1, :], o_ps1, recipb, mult)

                    if held is not None:
                        _emit_o2_late(held)
                    pending.append((nb, o8_sb))
                if pending:
                    _emit_o2_late(_emit_o2_early(pending.pop(0)))

    _strip_pe_self_waits(nc)
    nc.compile()
    return nc


def _get_nc():
    if "nc" not in _CACHE:
        _CACHE["nc"] = _build_nc()
    return _CACHE["nc"]


def make_in_maps(x, w_theta, w_phi, w_g, w_o, u_theta, u_phi, u_g, u_o, gamma):
    import ml_dtypes

    e4 = ml_dtypes.float8_e4m3
    wt = _sn(w_theta, u_theta).T                                  # [512, 64]
    wp = _sn(w_phi, u_phi).T                                      # [512, 64]
    wtp = np.ascontiguousarray(
        np.concatenate([wt, wp], axis=1).astype(np.float16))      # [512, 128]
    wg = np.ascontiguousarray(
        (WG_SCALE * _sn(w_g, u_g).T).astype(e4))                  # [512, 256]
    wo = np.ascontiguousarray(
        (WO_SCALE * np.float32(np.asarray(gamma, np.float32))
         * _sn(w_o, u_o).T).astype(e4))                           # [256, 512]
    xf = np.asarray(x, np.float32).reshape(B, C, HW)
    x16 = xf.astype(np.float16)
    x8 = xf.astype(e4)
    return [
        {"x16": np.ascontiguousarray(x16[i]),
         "x8": np.ascontiguousarray(x8[i]),
         "wtp": wtp, "wg": wg, "wo": wo}
        for i in range(B)
    ]


def kernel(x, w_theta, w_phi, w_g, w_o, u_theta, u_phi, u_g, u_o, gamma):
    from concourse.bass_utils import run_bass_kernel_spmd

    in_maps = make_in_maps(
        x, w_theta, w_phi, w_g, w_o, u_theta, u_phi, u_g, u_o, gamma
    )
    nc = _get_nc()
    res = run_bass_kernel_spmd(nc, in_maps, core_ids=list(range(B)))
    out = np.stack([np.asarray(r["out"], np.float32) for r in res.results],
                   axis=0)
    return out.reshape(B, C, H, W)


# revision 17
# speedup vs baseline: 1.0245x; 1.0245x over previous
"""Self-attention (SAGAN-style, spectral-normalized 1x1 convs) on 8 TRN2 cores.

Contract: kernel(**inputs) takes the FULL unsharded inputs
(x [8,512,64,64], weights, power-iteration u vectors, gamma) and returns
the FULL output [8,512,64,64] (float32).

Sharding: data-parallel over batch B=8 -> one batch element per core.
Each core runs the complete attention block for its element; no
collectives are needed.

Per-core math (C=512, HW=4096, M=HW/4=1024):
    theta = sn(w_theta) @ x          [64, 4096]
    phi   = maxpool2(sn(w_phi) @ x)  [64, 1024]
    g     = maxpool2(sn(w_g)   @ x)  [256, 1024]
    sT[m,n] = sum_c phi[c,m] theta[c,n]
    beta  = softmax over m  (exp without max-subtraction: logits span
            ~+-51, safe in fp32/bf16; normalization applied to o)
    o     = (g @ exp(sT)) * (1/sum)  [256, 4096]
    out   = gamma * (sn(w_o) @ o) + x

Precision plan (measured 1.07e-2 output rel err vs the 2e-2 gate):
 - logit path fp16 (x16, theta, phi); E/g/gT bf16; fp32 PSUM
 - g-projection fp8 e4m3 DoubleRow (k=256/matmul); wg pre-scaled by 8
   (e4m3-normal range), compensated exactly by the 8.0-valued
   ones-matmul that makes the softmax denominators
 - o stored e4m3; out-projection fp8 e4m3 DoubleRow with wo pre-scaled
   by 512*gamma; 1/512 rides in the residual ops' free scale slots
 - residual + output fp16 (host upcasts to fp32)

Schedule notes (from perfetto/ntff analysis of earlier versions):
 - FD=512 matmuls pipeline at ~216ns start-to-start (incl implicit
   LDWEIGHTS), so wall time ~ matmul slot count; both big k>=256
   contractions (g-proj, out-proj) use fp8 DoubleRow to halve slots.
 - sT matmuls are plain full-row k=128: theta/phi partition rows 64:128
   are ZEROED once, so each k=64 product rides a full matmul without
   tile_position packing (measured: packed pairs serialize their
   LDWEIGHTS on row-group conflicts and cost ~2.3x a plain slot).
 - exp starts as soon as theta/phi are done (~16us): sT units for
   blocks 0-3 are WOVEN between the g-projection/transpose blocks, so
   the 36us ACT exp stream (the #2 floor of this kernel) overlaps the
   whole projection tail. x16 rides sync+scalar DMA rings, x8+weights
   ride the gpsimd ring in parallel; all DMA dispatches lead the program.
 - maxpool is 2-stage (one 1x PSUM-read max + one 2x SBUF max) instead
   of copy+3 maxes: DVE proj load drops ~2.5x.
 - softmax sums: 3-level DVE tree + ONE 8.0-ones-matmul per block;
   reciprocal_approx_fast; sums sit between the two o-matmul halves.
 - out-projection of block nb-1 is emitted split: PE matmuls + 2 DVE
   fused residuals early (fills exp waits), ACT copy-scale + GPSIMD
   adds + one out-DMA per block later.

PE->PE self-waits are stripped (PE->PSUM write port is FIFO) and bacc's
generate_event_semaphores legalizes the 1-wait ISA limit.

The spectral-norm power-iteration only involves [1,64]x[64,512]
matvecs, so it runs on the host in float32; gamma is folded into w_o.
"""

import numpy as np

B, C, H, W = 8, 512, 64, 64
HW = H * W            # 4096
M = HW // 4           # 1024 (pooled spatial)
C8 = C // 8           # 64
C2 = C // 2           # 256
P = 128               # SBUF partitions
KC = C // P           # 4 k-chunks for C-contraction
FB = 512              # free-dim block
NB = HW // FB         # 8 n-blocks
MC = M // P           # 8 m-chunks
WG_SCALE = 8.0        # host wg scale (e4m3 normal range), cancelled by
                      # using this value in the ones-matmul
WO_SCALE = 512.0      # host wo scale (e4m3 normal range), cancelled in
                      # the residual ops' scale slots
EPS = 1e-12

_CACHE = {}


def _sn(w, u):
    """Host-side spectral norm (eval-mode power iteration), float32."""
    w = np.asarray(w, np.float32)
    u = np.asarray(u, np.float32)
    v = u @ w
    v = v / max(np.float32(np.linalg.norm(v)), np.float32(EPS))
    u2 = v @ w.T
    u2 = u2 / max(np.float32(np.linalg.norm(u2)), np.float32(EPS))
    sv = np.float32((v @ w.T @ u2.T)[0, 0])
    return w / sv


def _strip_pe_self_waits(nc):
    """Remove S[PE]-waits from PE matmuls: PE->PE deps are ordered by the
    engine queue + FIFO PSUM write port, and matmuls only have one ISA
    wait slot."""
    import concourse.mybir as mybir

    for f in nc.m.functions:
        for blk in f.blocks:
            for inst in blk.instructions:
                if not isinstance(inst, mybir.InstMatmult):
                    continue
                si = inst.sync_info
                kept = [w for w in si.on_wait
                        if not (w.ant_name or "").startswith("PE_")]
                if len(kept) != len(si.on_wait):
                    si.on_wait = kept
                    inst.sync_info = si


def _build_nc():
    import concourse.bass as bass
    import concourse.mybir as mybir
    import concourse.tile as tile
    from concourse import bacc
    from concourse.masks import make_identity

    fp32 = mybir.dt.float32
    fp16 = mybir.dt.float16
    bf16 = mybir.dt.bfloat16
    fp8 = mybir.dt.float8e4
    DR = mybir.MatmulPerfMode.DoubleRow
    Exp = mybir.ActivationFunctionType.Exp
    mult = mybir.AluOpType.mult
    add = mybir.AluOpType.add
    mx = mybir.AluOpType.max

    nc = bacc.Bacc()
    x16_d = nc.dram_tensor("x16", [C, HW], fp16, kind="ExternalInput").ap()
    x8_d = nc.dram_tensor("x8", [C, HW], fp8, kind="ExternalInput").ap()
    wtp_d = nc.dram_tensor("wtp", [C, P], fp16, kind="ExternalInput").ap()
    wg_d = nc.dram_tensor("wg", [C, C2], fp8, kind="ExternalInput").ap()
    wo_d = nc.dram_tensor("wo", [C2, C], fp8, kind="ExternalInput").ap()
    out_d = nc.dram_tensor("out", [C, HW], fp16, kind="ExternalOutput").ap()

    x16_r = x16_d.rearrange("(kc p) n -> p kc n", p=P)
    x8_r = x8_d.rearrange("(kc p) n -> p kc n", p=P)
    out_r = out_d.rearrange("(ig p) n -> p ig n", p=P)

    with tile.TileContext(nc) as tc:
        with (
            tc.tile_pool(name="sb", bufs=1) as sb,
            tc.tile_pool(name="psS", bufs=2, space="PSUM") as psS,
        ):
            # ---- persistent tiles ----
            x16 = sb.tile([P, KC, HW], fp16)
            x8 = sb.tile([P, KC, HW], fp8)
            wtp = sb.tile([P, KC, P], fp16)
            wg8 = sb.tile([P, KC, C2], fp8)
            wo8 = sb.tile([P, 2, C], fp8)
            theta_sb = sb.tile([P, HW], fp16)             # rows 64:128 zero
            phi2 = sb.tile([P, NB, 4, 32], fp16)          # rows 64:128 zero
            g2 = sb.tile([P, 2, M], bf16)                 # pooled, = 8*g
            gT_sb = sb.tile([P, MC, C2], bf16)            # [m-part, mc, c]
            identity = sb.tile([P, P], bf16)
            ones_mat = sb.tile([P, P], bf16)              # value WG_SCALE

            phi_flat = phi2.rearrange("p a b c -> p (a b c)")
            g4 = g2.rearrange("p cg (fb h2 w2) -> p cg fb h2 w2", h2=4, w2=32)

            # ---- input DMAs lead the program: x16 on the sync+scalar
            # rings, x8 + weights on the gpsimd ring (parallel) ----
            nc.sync.dma_start(wtp, wtp_d.rearrange("(kc p) i -> p kc i", p=P))
            for q in range(NB):
                sl = slice(q * FB, (q + 1) * FB)
                eng = nc.sync if q % 2 == 0 else nc.scalar
                eng.dma_start(x16[:, :, sl], x16_r[:, :, sl])
            nc.gpsimd.dma_start(wg8, wg_d.rearrange("(kc p) i -> p kc i", p=P))
            nc.gpsimd.dma_start(wo8, wo_d.rearrange("(cg p) i -> p cg i", p=P))
            for q in range(NB // 2):
                fsl = slice(2 * q * FB, (2 * q + 2) * FB)
                nc.gpsimd.dma_start(x8[:, :, fsl], x8_r[:, :, fsl])

            # ---- constants; zero the unused partition halves so sT can
            # run plain full-row k=128 matmuls ----
            nc.vector.memset(ones_mat, WG_SCALE)
            nc.vector.memset(theta_sb[C8:, :], 0.0)
            nc.vector.memset(phi2[C8:], 0.0)
            with tc.tile_pool(name="psW", bufs=1, space="PSUM") as psW:
                # junk matmuls keep the PE busy during the DMA head so the
                # HAM clock-gate opens before the real matmul stream
                wps = psW.tile([P, P], fp32, tag="warm")
                for _ in range(10):
                    nc.tensor.matmul(wps, lhsT=ones_mat, rhs=ones_mat,
                                     start=True, stop=True)
            ident_raw = sb.tile([P, P], fp32)
            make_identity(nc, ident_raw)
            nc.scalar.copy(identity, ident_raw)

            # ---------- attention front half: sT unit emitter ----------
            # expT tiles for blocks 0-3 are produced during the projection
            # weave; bufs=5 keeps enough alive until their o-matmuls run.
            exp_tiles = {}

            def st_pair(nb, mc2):
                """Two sT chunks: full-row k=128 matmuls (rows 64:128 are
                zeros) into one PSUM tile + a single FD-1024 exp."""
                if mc2 == 0:
                    exp_tiles[nb] = sb.tile([P, MC, FB], bf16, tag="expT",
                                            bufs=5, name=f"expT{nb}")
                expT = exp_tiles[nb]
                nsl = slice(nb * FB, (nb + 1) * FB)
                ps = psS.tile([P, 2, FB], fp32, tag="sT")
                for j in range(2):
                    mc = 2 * mc2 + j
                    nc.tensor.matmul(
                        ps[:, j, :],
                        lhsT=phi_flat[:, mc * P:(mc + 1) * P],
                        rhs=theta_sb[:, nsl],
                        start=True, stop=True,
                    )
                nc.scalar.activation(
                    expT[:, 2 * mc2:2 * mc2 + 2, :].rearrange("p a b -> p (a b)"),
                    ps.rearrange("p a b -> p (a b)"), Exp,
                )

            def pool2(ps2, dst, base, dt):
                """3-op 2x2 maxpool of a [*, 2, FB] PSUM view into dst
                [*, 2, 4, 32]: PSUM copy (wr=0), max vs PSUM (wr=1) -- only
                one PSUM operand per op -- then a 2x-rate SBUF hr-max."""
                v = ps2.rearrange("p a (h2 hr w2 wr) -> p a h2 hr w2 wr",
                                  hr=2, w2=32, wr=2)
                s1 = sb.tile([P, 2, 4, 2, 32], dt, tag="pool1", bufs=3,
                             name="s1")[base:]
                nc.vector.tensor_copy(s1, v[:, :, :, :, :, 0])
                nc.vector.tensor_tensor(s1, s1, v[:, :, :, :, :, 1], mx)
                nc.vector.tensor_tensor(dst, s1[:, :, :, 0, :],
                                        s1[:, :, :, 1, :], mx)

            # ---------- theta/phi projection (x16-chunk paced) ----------
            with tc.tile_pool(name="psB", bufs=2, space="PSUM") as psB:
                for fb2 in range(NB // 2):
                    ps = psB.tile([P, 2, FB], fp32, tag="proj", name="ps")
                    for half in range(2):
                        for kc in range(KC):
                            nc.tensor.matmul(
                                ps[:, half, :],
                                lhsT=wtp[:, kc, :],
                                rhs=x16[:, kc, (2 * fb2 + half) * FB:(2 * fb2 + half + 1) * FB],
                                start=(kc == 0), stop=(kc == KC - 1),
                            )
                    th = ps[:C8].rearrange("p a b -> p (a b)")
                    thsl = slice(2 * fb2 * FB, (2 * fb2 + 2) * FB)
                    nc.scalar.copy(theta_sb[:C8, thsl], th)
                    pool2(ps[C8:], phi2[:C8, 2 * fb2:2 * fb2 + 2], C8, fp16)

            # ---------- g-projection + transposes, WOVEN with the sT
            # pairs of blocks 0-3 so the exp stream starts immediately;
            # the single-buffered g pool's wait-for-pool gaps are filled
            # by the sT matmuls ----------
            with (
                tc.tile_pool(name="psA", bufs=1, space="PSUM") as psA,
                tc.tile_pool(name="psT", bufs=1, space="PSUM") as psT,
            ):
                def g_block(fb2, cg):
                    ps = psA.tile([P, 2, FB], fp32, tag="proj", name="psg")
                    for half in range(2):
                        hsl = slice((2 * fb2 + half) * FB,
                                    (2 * fb2 + half + 1) * FB)
                        for qq in range(2):
                            nc.tensor.matmul(
                                ps[:, half, :],
                                lhsT=wg8[:, 2 * qq:2 * qq + 2, cg * P:(cg + 1) * P],
                                rhs=x8[:, 2 * qq:2 * qq + 2, hsl],
                                start=(qq == 0), stop=(qq == 1),
                                perf_mode=DR,
                            )
                    pool2(ps, g4[:, cg, 2 * fb2:2 * fb2 + 2], 0, bf16)

                def tr_block(fb2):
                    for mc in (2 * fb2, 2 * fb2 + 1):
                        pt = psT.tile([P, 2, P], bf16, tag="tr")
                        for cg in range(2):
                            nc.tensor.transpose(
                                pt[:, cg, :], g2[:, cg, mc * P:(mc + 1) * P],
                                identity,
                            )
                        nc.scalar.copy(gT_sb[:, mc, :],
                                       pt.rearrange("p a b -> p (a b)"))

                pairs = [(nb, mc2) for nb in range(4) for mc2 in range(MC // 2)]
                ui = 0
                for fb2 in range(NB // 2):
                    for cg in range(2):
                        for _ in range(2):
                            st_pair(*pairs[ui]); ui += 1
                        g_block(fb2, cg)
                    tr_block(fb2)
                assert ui == len(pairs)

            # ---------- attention back half ----------
            with (
                tc.tile_pool(name="psO", bufs=2, space="PSUM") as psO,
                tc.tile_pool(name="psO2", bufs=2, space="PSUM") as psO2,
            ):
                pending = []

                def _emit_o2_early(item):
                    """Out-projection matmuls of block jnb (4 fp8-DR, k=256)
                    + the 2 DVE fused residuals."""
                    jnb, jo8 = item
                    jsl = slice(jnb * FB, (jnb + 1) * FB)
                    ot4 = sb.tile([P, 4, FB], fp16, tag="out", bufs=2,
                                  name="ot4")
                    o2s = []
                    for ig in range(4):
                        o2 = psO2.tile([P, FB], fp32, tag="o2", name="o2")
                        nc.tensor.matmul(
                            o2,
                            lhsT=wo8[:, :, ig * P:(ig + 1) * P],
                            rhs=jo8,
                            start=True, stop=True, perf_mode=DR,
                        )
                        if ig < 2:
                            nc.vector.scalar_tensor_tensor(
                                ot4[:, ig, :], o2, 1.0 / WO_SCALE,
                                x16[:, ig, jsl], op0=mult, op1=add)
                        else:
                            o2s.append(o2)
                    return (jnb, ot4, o2s)

                def _emit_o2_late(item):
                    """ACT copy-with-scale + GPSIMD residual adds for ig 2,3
                    and the single out-DMA."""
                    jnb, ot4, o2s = item
                    jsl = slice(jnb * FB, (jnb + 1) * FB)
                    for ig, o2 in zip((2, 3), o2s):
                        st = sb.tile([P, FB], fp32, tag="stage", bufs=4,
                                     name="st")
                        nc.scalar.mul(st, o2, 1.0 / WO_SCALE)
                        nc.gpsimd.tensor_tensor(ot4[:, ig, :], st,
                                                x16[:, ig, jsl], add)
                    nc.sync.dma_start(out_r[:, :, jsl], ot4)

                for nb in range(NB):
                    held = None
                    if nb >= 4:
                        st_pair(nb, 0)
                        st_pair(nb, 1)
                        held = _emit_o2_early(pending.pop(0)) if pending else None
                        st_pair(nb, 2)
                        st_pair(nb, 3)
                    else:
                        held = _emit_o2_early(pending.pop(0)) if pending else None
                    expT = exp_tiles.pop(nb)

                    # sum over m: 3-level DVE tree, then ONE ones-matmul
                    # (value 8.0) sums the partitions; output rows all hold
                    # the sum -> broadcast-ready
                    part = sb.tile([P, 4, FB], bf16, tag="part", bufs=2)
                    nc.vector.tensor_tensor(
                        part.rearrange("p a b -> p (a b)"),
                        expT[:, 0:4, :].rearrange("p a b -> p (a b)"),
                        expT[:, 4:8, :].rearrange("p a b -> p (a b)"), add)
                    nc.vector.tensor_tensor(
                        part[:, 0:2, :].rearrange("p a b -> p (a b)"),
                        part[:, 0:2, :].rearrange("p a b -> p (a b)"),
                        part[:, 2:4, :].rearrange("p a b -> p (a b)"), add)
                    nc.vector.tensor_tensor(part[:, 0, :], part[:, 0, :],
                                            part[:, 1, :], add)

                    # o[c, n] = sum_m gT[m, c] expT[m, n]; the sum-matmul
                    # sits between the cg halves so the reciprocal is ready
                    # when the first half finishes accumulating
                    o8_sb = sb.tile([P, 2, FB], fp8, tag="o8", bufs=2)
                    o_ps0 = psO.tile([P, FB], fp32, tag="o_ps", name="o0")
                    for mc in range(MC):
                        nc.tensor.matmul(
                            o_ps0,
                            lhsT=gT_sb[:, mc, 0:P],
                            rhs=expT[:, mc, :],
                            start=(mc == 0), stop=(mc == MC - 1),
                        )
                    o_ps1 = psO.tile([P, FB], fp32, tag="o_ps", name="o1")
                    for mc in range(MC):
                        nc.tensor.matmul(
                            o_ps1,
                            lhsT=gT_sb[:, mc, P:C2],
                            rhs=expT[:, mc, :],
                            start=(mc == 0), stop=(mc == MC - 1),
                        )
                    sum_ps = psS.tile([P, 2, FB], fp32, tag="sT",
                                      name="sum_ps")[:, 0, :]
                    nc.tensor.matmul(sum_ps, lhsT=ones_mat, rhs=part[:, 0, :],
                                     start=True, stop=True)
                    recipb = sb.tile([P, FB], fp32, tag="recipb", bufs=2)
                    nc.vector.reciprocal_approx_fast(recipb, sum_ps)
                    nc.vector.tensor_tensor(o8_sb[:, 0, :], o_ps0, recipb, mult)
                    nc.vector.tensor_tensor(o8_sb[:, 1, :], o_ps1, recipb, mult)

                    if held is not None:
                        _emit_o2_late(held)
                    pending.append((nb, o8_sb))
                if pending:
                    _emit_o2_late(_emit_o2_early(pending.pop(0)))

    _strip_pe_self_waits(nc)
    nc.compile()
    return nc


def _get_nc():
    if "nc" not in _CACHE:
        _CACHE["nc"] = _build_nc()
    return _CACHE["nc"]


def make_in_maps(x, w_theta, w_phi, w_g, w_o, u_theta, u_phi, u_g, u_o, gamma):
    import ml_dtypes

    e4 = ml_dtypes.float8_e4m3
    wt = _sn(w_theta, u_theta).T                                  # [512, 64]
    wp = _sn(w_phi, u_phi).T                                      # [512, 64]
    wtp = np.ascontiguousarray(
        np.concatenate([wt, wp], axis=1).astype(np.float16))      # [512, 128]
    wg = np.ascontiguousarray(
        (WG_SCALE * _sn(w_g, u_g).T).astype(e4))                  # [512, 256]
    wo = np.ascontiguousarray(
        (WO_SCALE * np.float32(np.asarray(gamma, np.float32))
         * _sn(w_o, u_o).T).astype(e4))                           # [256, 512]
    xf = np.asarray(x, np.float32).reshape(B, C, HW)
    x16 = xf.astype(np.float16)
    x8 = xf.astype(e4)
    return [
        {"x16": np.ascontiguousarray(x16[i]),
         "x8": np.ascontiguousarray(x8[i]),
         "wtp": wtp, "wg": wg, "wo": wo}
        for i in range(B)
    ]


def kernel(x, w_theta, w_phi, w_g, w_o, u_theta, u_phi, u_g, u_o, gamma):
    from concourse.bass_utils import run_bass_kernel_spmd

    in_maps = make_in_maps(
        x, w_theta, w_phi, w_g, w_o, u_theta, u_phi, u_g, u_o, gamma
    )
    nc = _get_nc()
    res = run_bass_kernel_spmd(nc, in_maps, core_ids=list(range(B)))
    out = np.stack([np.asarray(r["out"], np.float32) for r in res.results],
                   axis=0)
    return out.reshape(B, C, H, W)
